# revision 21
# baseline (speedup 1.0000x reference)
"""Bass/Trainium2 kernel for nn_LowRankLoss.

Reference computation:
  m      = mean(feat, axis=1)                      # [n, h, w], channel mean
  normed = m / ||m||_F (per sample)
  rank   = #(singular values of normed > 0)        # [n]
  loss   = sum(max(0, -(rank1 - rank2))) / n

The memory-bound part (target_regime=memory) is the channel-mean reduction
over two [128, 256, 32, 64] f32 tensors (512 MiB total). That runs on 8
NeuronCores, data-parallel over the batch dim (16 samples/core). The device
returns per-sample channel sums [n, 2048]; the tiny per-sample SVDs
(128 matrices of 32x64) and the scalar loss are finished on host.

Device design per core (per input tensor, viewed [NS=16, 2, 128, F=2048]):
  - Two contiguous 1 MiB DMAs per sample (8 KiB per-partition
    descriptors), riding the two HWDGE rings: t0 on sync, t1 on scalar.
    HWDGE has no SBUF descriptor ring, so the SDMA-7/15 descriptor-fetch
    port contention that intermittently drags one engine ~20% under
    SWDGE (and with statically balanced work, the whole stream) cannot
    bite; measured back-to-back runs agree within 0.05%. The scalar ring
    is rotated one slot early (t1 of global sample k+1 is issued while
    sample k streams; t1 of sample 0 prepends the sync queue), so each
    sample's t1 lands ~5 us before its t0 and only the h0 matmuls
    remain after the final input byte.
  - TensorE reduces all 256 channels per sample directly from the raw
    input tiles: tiles are declared f32r and the DRAM source is bitcast
    (same bits; the PE truncates the mantissa while streaming 1
    cycle/row vs 4 for fp32), so there is no VectorE fold at all. The
    one-hot stationary S_m [128, 16] puts sample s in PSUM row s%16
    while other rows accumulate +0; per sample h1's four bank-chunk
    matmuls run first, then h0's (data always ready). One 16-sample
    group per tensor: raw accumulates in one 4-bank PSUM tile, rect in
    four per-bank tiles (32-matmul chains per 512-col bank) so each
    drain copy depends only on its own chain's stop-matmul -- PSUM dep
    tracking is whole-tile-conservative otherwise.
  - raw group: acc -> SBUF copy (DVE) -> 128 KiB DMA out on the
    otherwise-idle SWDGE queue (an output wait on the input rings would
    stall HWDGE descriptor generation).
  - rect group (the drain): per-bank copies alternate ACT/DVE into four
    separate tiles (issued before any output DMA so ACT's queue is pure
    copies), then four chunk output DMAs split across both HWDGE rings.
    After the last input byte only: 4 h0 matmuls -> 2+2 parallel copies
    -> 2+2 parallel descriptor-gens -> 16 KiB output bytes remain.
  - The ~8.5 us after the last output DMA is the fixed framework
    epilogue (DMA-lane drains, all-engine barriers, a constant ~4.2 us
    quiesce, per-engine semaphore-file resets): kernel-independent.
f32r truncates the data mantissa (~1e-4 rel err on the channel sums),
far below what could flip a singular-value-positivity count (min sigma
~2e-2 here).
"""

import numpy as np

N_CORES = 8
NS = 16           # samples per core
C = 256           # channels
H, W = 32, 64
F = H * W         # 2048 spatial
CB = 2            # channel halves
P = 128           # partitions
SG = 16           # samples per PSUM group (one group per tensor)
NB = 4            # matmuls per half-sample (N=512 PSUM bank limit)
BN = F // NB      # 512

_CACHE = {}


def _build_nc():
    import concourse.bacc as bacc
    import concourse.mybir as mybir
    import concourse.tile as tile

    nc = bacc.Bacc(None, target_bir_lowering=False)
    f32 = mybir.dt.float32
    f32r = mybir.dt.float32r

    x_raw = nc.dram_tensor("x_raw", [NS, CB, P, F], f32, kind="ExternalInput")
    x_rect = nc.dram_tensor("x_rect", [NS, CB, P, F], f32, kind="ExternalInput")
    out_raw = nc.dram_tensor("out_raw", [NS, F], f32, kind="ExternalOutput")
    out_rect = nc.dram_tensor("out_rect", [NS, F], f32, kind="ExternalOutput")

    with tile.TileContext(nc) as tc:
        with (
            tc.tile_pool(name="io", bufs=10) as pool,
            tc.tile_pool(name="small", bufs=2) as small,
            tc.tile_pool(name="psum", bufs=2, space="PSUM") as psum,
        ):
            tensors = ((x_raw, out_raw), (x_rect, out_rect))
            seq = [(ti, s) for ti in range(2) for s in range(NS)]

            # scalar-ring rotation: t1 of global sample k+1 is issued
            # during sample k; t1 of sample 0 heads the sync queue
            t1_pending = {}
            xt0 = tensors[0][0]
            t1f = pool.tile([P, F], f32r, tag="in1", name="t1f")
            nc.sync.dma_start(t1f[:], xt0[0, 1].bitcast(f32r), single_packet=True)
            t1_pending[0] = t1f

            # C[k, 16m + j] = 1 if j == m else 0; lhsT for sample s is the
            # [128, 16] slice C[:, 16m:16m+16] with m = s % 16.
            s_np = np.zeros((P, SG * SG), np.float32)
            for m in range(SG):
                s_np[:, SG * m + m] = 1.0
            s_dram = nc.inline_tensor(s_np, name="s_const")
            s_stage = small.tile([P, SG * SG], f32, tag="stat_stage")
            nc.sync.dma_start(s_stage[:], s_dram[:])
            S = small.tile([P, SG * SG], f32r, tag="stat")
            nc.vector.tensor_copy(S[:], s_stage[:])

            # raw: one whole-group tile (4 banks). rect: four per-bank
            # tiles (4 banks) so each drain copy's dependency is its own
            # chain's stop-matmul — whole-tile-conservative dep tracking
            # otherwise holds every copy until the final stop.
            acc0 = psum.tile([SG, F], f32, tag="acc", bufs=1, name="acc0")
            acc_full = {0: acc0}
            acc_of = {
                0: [acc0[:, j * BN : (j + 1) * BN] for j in range(NB)],
                1: [
                    psum.tile([SG, BN], f32, tag=f"accc{j}", bufs=1,
                              name=f"accc{j}")[:]
                    for j in range(NB)
                ],
            }

            for k, (ti, m) in enumerate(seq):
                xt, ot = tensors[ti]
                accb = acc_of[ti]
                # next sample's t1 rides the scalar ring one slot early
                if k + 1 < len(seq):
                    nti, nm = seq[k + 1]
                    t1n = pool.tile([P, F], f32r, tag="in1", name="t1n")
                    nc.scalar.dma_start(t1n[:], tensors[nti][0][nm, 1].bitcast(f32r), single_packet=True)
                    t1_pending[k + 1] = t1n
                t0 = pool.tile([P, F], f32r, tag="in0")
                nc.sync.dma_start(t0[:], xt[m, 0].bitcast(f32r), single_packet=True)
                t1 = t1_pending.pop(k)
                # h1 first (its tile arrived a slot ago), h0 gates the chain
                for h, t in ((1, t1), (0, t0)):
                    for j in range(NB):
                        nc.tensor.matmul(
                            accb[j],
                            S[:, SG * m : SG * m + SG],
                            t[:, j * BN : (j + 1) * BN],
                            start=(m == 0 and h == 1),
                            stop=(m == SG - 1 and h == 0),
                        )
                if m != SG - 1:
                    continue
                if ti == 0:
                    osb = small.tile([SG, F], f32, tag="osb")
                    nc.vector.tensor_copy(osb[:], acc_full[0][:])
                    # SWDGE queue is idle: its copy-wait stalls nothing
                    nc.gpsimd.dma_start(ot[:], osb[:])
                else:
                    # chunked drain: copies issued before any output DMA
                    # (ACT otherwise wedges descriptor generation between
                    # its two copies), four separate tiles (a shared tile
                    # serializes copies against the output DMA reads)
                    osbcs = []
                    for j in range(NB):
                        osbc = small.tile([SG, BN], f32, tag="osbc", bufs=4,
                                          name=f"osbc{j}")
                        if j % 2 == 0:
                            nc.scalar.copy(osbc[:], accb[j])
                        else:
                            nc.vector.tensor_copy(osbc[:], accb[j])
                        osbcs.append(osbc)
                    for j in range(NB):
                        c0, c1 = j * BN, (j + 1) * BN
                        eng = nc.sync if j % 2 == 0 else nc.scalar
                        eng.dma_start(ot[:, c0:c1], osbcs[j][:])

    nc.compile()
    return nc


def _device_channel_sums(raw, rect, trace=False):
    """Run the bass kernel on 8 cores; return (sums_raw, sums_rect) [128, 2048]
    and the BassKernelResults."""
    from concourse.bass_utils import run_bass_kernel_spmd

    if "nc" not in _CACHE:
        _CACHE["nc"] = _build_nc()
    nc = _CACHE["nc"]

    raw5 = raw.reshape(N_CORES, NS, CB, P, F)
    rect5 = rect.reshape(N_CORES, NS, CB, P, F)
    in_maps = [{"x_raw": raw5[i], "x_rect": rect5[i]} for i in range(N_CORES)]
    res = run_bass_kernel_spmd(nc, in_maps, list(range(N_CORES)), trace=trace)

    sums_raw = np.concatenate([res.results[i]["out_raw"] for i in range(N_CORES)])
    sums_rect = np.concatenate([res.results[i]["out_rect"] for i in range(N_CORES)])
    return sums_raw, sums_rect, res


def _rank_from_sums(sums):
    # channel mean (exact: /256 is a power of two), normalize, svd, count
    m = (sums / np.float32(C)).astype(np.float32)
    nrm = np.linalg.norm(m, axis=1, keepdims=True)
    normed = (m / nrm).reshape(-1, H, W)
    s = np.linalg.svd(normed.astype(np.float32), compute_uv=False)
    return (s > 0.0).sum(axis=1).astype(np.float32)


def kernel(raw_feat, rectified_feat, trace=False):
    raw = np.ascontiguousarray(np.asarray(raw_feat, dtype=np.float32))
    rect = np.ascontiguousarray(np.asarray(rectified_feat, dtype=np.float32))

    sums_raw, sums_rect, res = _device_channel_sums(raw, rect, trace=trace)
    _CACHE["last_results"] = res
    _CACHE["last_sums"] = (sums_raw, sums_rect)

    rank1 = _rank_from_sums(sums_raw)
    rank2 = _rank_from_sums(sums_rect)
    loss = np.maximum(np.float32(0.0), -(rank1 - rank2))
    loss = loss.sum(dtype=np.float32) / np.float32(raw.shape[0])
    return np.asarray(loss, dtype=np.float32)
